# revision 43
# baseline (speedup 1.0000x reference)
"""Trainium2 Bass kernel for nn_AutoSlicingModel (segment_reduce).

Computation (per batch item):
  stmt_emb[s]  = mean of hidden_states over the 8 contiguous tokens of statement s
  var_emb      = mean of hidden_states rows at variables_ids (8 occurrences)
  paired[s]    = [stmt_emb[s], var_emb]           (2H = 2048)
  back_preds   = SliceMLP_back(paired[0:128])     (3-layer MLP, gelu/gelu/sigmoid)
  fwd_preds    = SliceMLP_fwd (paired[129:256])
  out          = concat([back_preds, fwd_preds])  -> [B, 255]

Distribution: data-parallel over batch B=64 across 8 NeuronCores (8 items/core),
MLP weights replicated, no cross-core communication; host concatenates.

Device strategy (v2):
  * X is host-pretransposed to [feature-partition, token] layout so the
    segment pooling is a free-dim strided sum on the VECTOR engine (DVE),
    not the tensor engine.  Pooling output lands directly in the
    [feature, (item, seg)] orientation the MLP matmuls consume.
  * The tensor engine runs only the MLP.  Layers run in fp8 (e4m3) with
    DoubleRow matmuls (K=256 per pass) for ~1.7x bf16 throughput; PSUM
    accumulates in fp32.  Weights are scaled x16 on host to stay in the
    e4m3 normal range; activations fold the inverse scales exactly.
  * var_emb's layer-1 contribution (var @ W1b, constant per item) is
    computed once per iteration as b1tT[item, f] (a K=128-per-chunk
    matmul with the 8-item varb block as the stationary operand), then
    injected into each layer-1 PSUM group via one extra K=8 matmul
    against a constant item-indicator matrix -- so layer-1's gelu is a
    single 512-wide activation with a per-feature bias.
  * Weights are DMA'd once (outside the repeat loop); only X, var_x and
    outputs move per iteration.  Steady-state target is the HBM roofline
    (~33.5 MB/core of bf16 X at ~358 GB/s ~= 94 us).
"""

import sys

if "/opt/trn_rl_repo" not in sys.path:
    sys.path.insert(0, "/opt/trn_rl_repo")

from contextlib import ExitStack

import ml_dtypes
import numpy as np

import concourse.bacc as bacc
import concourse.bass as bass
import concourse.mybir as mybir
import concourse.tile as tile
from concourse.bass_utils import run_bass_kernel_spmd

BF16 = mybir.dt.bfloat16
FP8 = mybir.dt.float8e4
F32 = mybir.dt.float32
NPBF16 = np.dtype(ml_dtypes.bfloat16)
NPFP8 = np.dtype(ml_dtypes.float8_e4m3)

B, T, H, S, V = 64, 2048, 1024, 256, 8
VAR_LINE = 128
NCORES = 8
IPC = B // NCORES        # items per core = 8
NQUAD = IPC // 4         # quads per core = 2
NK = H // 128            # 8 feature chunks
TPS = T // S             # tokens per statement = 8
BR = ("back", "fwd")

WSC = 16.0               # host weight scale (keeps fp8 weights in normal range)
# pooled sums are 8x the means; stmt and var sums hit layer 1 unscaled, so
# layer-1 PSUM carries 8*WSC = 128x the true pre-activation.
SC1 = 8.0 * WSC
SC2 = WSC                # h1 is true-scale, w2 is x16
SC3 = WSC

ACT = mybir.ActivationFunctionType
DR = mybir.MatmulPerfMode.DoubleRow

# Timing-experiment knob (see ab_time.py): "cce" is the real kernel.
ACCUM_MODE = "cce"


def _emit(ctx: ExitStack, tc: "tile.TileContext", out_ap: bass.AP, ins: dict,
          repeat: int = 1):
    nc = tc.nc

    consts = ctx.enter_context(tc.tile_pool(name="consts", bufs=1))
    work = ctx.enter_context(tc.tile_pool(name="work", bufs=1))
    psum = ctx.enter_context(tc.tile_pool(name="psum", bufs=1, space="PSUM"))

    def const_tile(name, shape, dtype):
        t = consts.tile(shape, dtype, name=name, tag=name)
        nc.sync.dma_start(t[:], ins[name][:])
        return t

    # Weights/constants: DMA'd once, resident for all repeats.
    C = {}
    C["ind"] = const_tile("ind", [8, NQUAD * 512], BF16)
    for br in BR:
        C[f"w1a_{br}"] = const_tile(f"w1a_{br}", [128, 4, 2, H], FP8)
        C[f"w1b_{br}"] = const_tile(f"w1b_{br}", [128, NK, H], BF16)
        C[f"w2_{br}"] = const_tile(f"w2_{br}", [128, 4, 2, H], FP8)
        C[f"w3_{br}"] = const_tile(f"w3_{br}", [128, 4, 2, 16], FP8)
        C[f"b1h_{br}"] = const_tile(f"b1h_{br}", [128, NK], F32)
        C[f"b2t_{br}"] = const_tile(f"b2t_{br}", [128, NK], F32)
        C[f"b3_{br}"] = const_tile(f"b3_{br}", [1, 1], F32)

    for _rep in range(repeat):
        _emit_once(nc, tc, work, psum, C, out_ap, ins)


def _emit_once(nc, tc, work, psum, C, out_ap, ins):
    # ---- var means -> per-item layer-1 bias b1tT[item, f] (scaled SC1) ----
    var_x = work.tile([128, NK * IPC * V], BF16, name="var_x", tag="var_x", bufs=1)
    nc.sync.dma_start(var_x[:], ins["var_x"][:])
    varb = work.tile([128, NK * IPC], BF16, name="varb", tag="varb", bufs=1)
    with nc.allow_low_precision("DVE reduces in fp32 internally; bf16 store ok"):
        nc.vector.tensor_reduce(
            varb[:].rearrange("p (c i) -> p c i", i=IPC),
            var_x[:].rearrange("p (c i v) -> p (c i) v", i=IPC, v=V),
            axis=mybir.AxisListType.X,
            op=mybir.AluOpType.add,
        )

    b1tT = {}
    for br in BR:
        b1tT[br] = work.tile([8, H], BF16, name="b1tT", tag=f"b1tT_{br}", bufs=1)
        for half in range(2):
            bp = psum.tile([8, 512], F32, name="bp", tag="b1t_ps", bufs=2)
            for kc in range(NK):
                nc.tensor.matmul(
                    bp[:],
                    varb[:, kc * IPC:(kc + 1) * IPC],
                    C[f"w1b_{br}"][:, kc, half * 512:(half + 1) * 512],
                    start=(kc == 0), stop=(kc == NK - 1),
                )
            nc.vector.tensor_copy(b1tT[br][:, half * 512:(half + 1) * 512], bp[:])

    logits = []
    for q in range(NQUAD):
        # ---- pooling: the DMA engines accumulate the 8 token-planes of each
        # segment into two fp8 slots (4 accumulating plane-pair DMAs, CCE adds
        # in fp32); the DVE only merges the two slots into the fp8 stmt tile.
        # X ships as fp8, halving HBM traffic vs bf16. ----
        stmt8 = {br: work.tile([128, NK, 512], FP8, name="stmt8",
                               tag=f"stmt8_{br}", bufs=2) for br in BR}
        # 3-slot hybrid pooling: HWDGE bulk-copies token-planes 0-2 into the
        # slots (no descriptor-gen cost on Pool), SWDGE CCE-accumulates planes
        # 3-7 on top (2KB-per-partition DMAs — the CCE accumulate cap), and
        # the DVE merges the three slots into the fp8 stmt tile.  Accumulate
        # chains are interleaved across the quad's items (k-major) so the Pool
        # sequencer never stalls on one chain's completion wait.
        accs = [work.tile([128, 3, NK * S], FP8, name="acc", tag="acc", bufs=8)
                for _ in range(4)]
        for iq in range(4):
            nc.sync.dma_start(accs[iq][:], ins["xc"][q * 4 + iq])
        if ACCUM_MODE != "none":
            op = (mybir.AluOpType.add if ACCUM_MODE == "cce"
                  else mybir.AluOpType.bypass)
            for k in range(5):
                for iq in range(4):
                    nc.gpsimd.dma_start(
                        accs[iq][:, k % 3, :], ins["xa"][q * 4 + iq, k],
                        accum_op=op,
                    )
        for iq in range(4):
            tmb = work.tile([128, NK * S], BF16, name="tmb", tag="tmb", bufs=2)
            av = accs[iq][:].rearrange("p h (c s) -> p h c s", c=NK)
            tv = tmb[:].rearrange("p (c s) -> p c s", c=NK)
            with nc.allow_low_precision("DVE adds in fp32 internally"):
                nc.vector.tensor_tensor(tmb[:], accs[iq][:, 0, :],
                                        accs[iq][:, 1, :], mybir.AluOpType.add)
                nc.vector.tensor_tensor(
                    stmt8["back"][:, :, iq * 128:(iq + 1) * 128],
                    tv[:, :, 0:128], av[:, 2, :, 0:128], mybir.AluOpType.add)
                nc.vector.tensor_tensor(
                    stmt8["fwd"][:, :, iq * 128:(iq + 1) * 128],
                    tv[:, :, 128:256], av[:, 2, :, 128:256],
                    mybir.AluOpType.add)

        # ---- MLP per branch ----
        for br in BR:
            h1 = work.tile([128, NK, 512], FP8, name="h1", tag="h1", bufs=2)
            for fc in range(NK):
                mp = psum.tile([128, 512], F32, name="mp", tag="mm_ps", bufs=2)
                for j in range(4):
                    nc.tensor.matmul(
                        mp[:],
                        C[f"w1a_{br}"][:, j, :, fc * 128:(fc + 1) * 128],
                        stmt8[br][:, 2 * j:2 * j + 2, :],
                        start=(j == 0), stop=False, perf_mode=DR,
                    )
                nc.tensor.matmul(
                    mp[:],
                    b1tT[br][:, fc * 128:(fc + 1) * 128],
                    C["ind"][:, q * 512:(q + 1) * 512],
                    start=False, stop=True, skip_group_check=True,
                )
                nc.scalar.activation(
                    h1[:, fc, :], mp[:], ACT.Gelu,
                    bias=C[f"b1h_{br}"][:, fc:fc + 1], scale=1.0 / SC1,
                )

            h2 = work.tile([128, NK, 512], FP8, name="h2", tag="h2", bufs=2)
            for fc in range(NK):
                mp = psum.tile([128, 512], F32, name="mp", tag="mm_ps", bufs=2)
                for j in range(4):
                    nc.tensor.matmul(
                        mp[:],
                        C[f"w2_{br}"][:, j, :, fc * 128:(fc + 1) * 128],
                        h1[:, 2 * j:2 * j + 2, :],
                        start=(j == 0), stop=(j == 3), perf_mode=DR,
                    )
                nc.scalar.activation(
                    h2[:, fc, :], mp[:], ACT.Gelu,
                    bias=C[f"b2t_{br}"][:, fc:fc + 1], scale=1.0 / SC2,
                )

            # layer 3 in DoubleRow with w3 zero-padded to M=16 output columns
            # (M=1 DR would fail the ISA's pair-dim AP shape check); only
            # psum row 0 is meaningful.
            lp = psum.tile([16, 512], F32, name="lp", tag="l3_ps", bufs=4)
            for j in range(4):
                nc.tensor.matmul(
                    lp[:], C[f"w3_{br}"][:, j, :, :], h2[:, 2 * j:2 * j + 2, :],
                    start=(j == 0), stop=(j == 3), perf_mode=DR,
                )
            logits.append((q, br, lp))

    # ---- sigmoid + output (deferred so the ACT gelu<->sigmoid table set
    # swaps happen once per iteration, not once per quad-branch) ----
    preds = work.tile([1, 2 * NQUAD * 512], F32, name="preds", tag="preds", bufs=2)
    pv = preds[:].rearrange("o (b q c) -> o b q c", b=2, q=NQUAD)
    for q, br, lp in logits:
        bi = 0 if br == "back" else 1
        nc.scalar.activation(pv[:, bi, q, :], lp[0:1, :], ACT.Sigmoid,
                             bias=C[f"b3_{br}"][:, :1], scale=1.0 / SC3)
    # two strided DMAs cover all 16 output strips
    ov = out_ap.rearrange("(q i) s -> q i s", q=NQUAD)
    nc.sync.dma_start(
        ov[:, :, 0:128],
        pv[:, 0].rearrange("o q (i s) -> o q i s", i=4),
    )
    nc.sync.dma_start(
        ov[:, :, 128:S - 1],
        pv[:, 1].rearrange("o q (i s) -> o q i s", i=4)[:, :, :, 1:128],
    )


# ------------------------- host-side preparation -------------------------

def _dr_pack(w):
    """[1024, M] -> DoubleRow stationary layout [128, 4, 2, M]:
    out[p, j, s, m] = w[128*(2j+s) + p, m]."""
    M = w.shape[1]
    return np.ascontiguousarray(w.reshape(4, 2, 128, M).transpose(2, 0, 1, 3))


def _prep_weights(inputs):
    g = {}
    ind = np.zeros((8, NQUAD * 512), np.float32)
    for qq in range(NQUAD):
        for i4 in range(4):
            ind[qq * 4 + i4, qq * 512 + i4 * 128:qq * 512 + (i4 + 1) * 128] = 1.0
    g["ind"] = ind.astype(NPBF16)
    for br in BR:
        w1 = np.asarray(inputs[f"{br}_w1"], np.float32)
        w2 = np.asarray(inputs[f"{br}_w2"], np.float32)
        w3 = np.asarray(inputs[f"{br}_w3"], np.float32)
        g[f"w1a_{br}"] = _dr_pack(w1[:H] * WSC).astype(NPFP8)
        # w1b: [128, kc, f] = w1[H + 128*kc + p, f] * WSC (bf16; feeds b1tT)
        g[f"w1b_{br}"] = np.ascontiguousarray(
            (w1[H:] * WSC).reshape(NK, 128, H).transpose(1, 0, 2)
        ).astype(NPBF16)
        g[f"w2_{br}"] = _dr_pack(w2 * WSC).astype(NPFP8)
        w3p = np.zeros((2 * H, 16), np.float32)
        w3p[:H, 0] = w3[:, 0] * WSC
        g[f"w3_{br}"] = _dr_pack(w3p[:H]).astype(NPFP8)
        g[f"b1h_{br}"] = np.ascontiguousarray(
            np.asarray(inputs[f"{br}_b1"], np.float32).reshape(NK, 128).T
        )
        g[f"b2t_{br}"] = np.ascontiguousarray(
            np.asarray(inputs[f"{br}_b2"], np.float32).reshape(NK, 128).T
        )
        g[f"b3_{br}"] = np.asarray(inputs[f"{br}_b3"], np.float32).reshape(1, 1)
    return g


_CACHE: dict = {}


def _build_program(repeat: int = 1):
    nc = bacc.Bacc("TRN2", target_bir_lowering=False, debug=False)
    shapes = {
        "xc": ([IPC, 128, 3, NK * S], FP8),
        "xa": ([IPC, 5, 128, NK * S], FP8),
        "var_x": ([128, NK * IPC * V], BF16),
        "ind": ([8, NQUAD * 512], BF16),
    }
    for br in BR:
        shapes[f"w1a_{br}"] = ([128, 4, 2, H], FP8)
        shapes[f"w1b_{br}"] = ([128, NK, H], BF16)
        shapes[f"w2_{br}"] = ([128, 4, 2, H], FP8)
        shapes[f"w3_{br}"] = ([128, 4, 2, 16], FP8)
        shapes[f"b1h_{br}"] = ([128, NK], F32)
        shapes[f"b2t_{br}"] = ([128, NK], F32)
        shapes[f"b3_{br}"] = ([1, 1], F32)
    aps = {
        name: nc.dram_tensor(name, shape, dt, kind="ExternalInput").ap()
        for name, (shape, dt) in shapes.items()
    }
    out = nc.dram_tensor("out", [IPC, S - 1], F32, kind="ExternalOutput").ap()
    with tile.TileContext(nc) as tc:
        with ExitStack() as ctx:
            _emit(ctx, tc, out, aps, repeat=repeat)
    nc.compile()
    return nc


def _make_in_maps(inputs):
    x = np.asarray(inputs["hidden_states"], np.float32)
    vids = np.asarray(inputs["variables_ids"], np.int64)
    sids = np.asarray(inputs["statements_ids"], np.int64)
    assert int(inputs["var_line"]) == VAR_LINE and int(inputs["num_statements"]) == S
    expect = np.tile(np.arange(T, dtype=np.int64) // TPS, (B, 1))
    assert np.array_equal(sids, expect), "statements_ids must be contiguous blocks"

    # x ships fp8, token-of-segment plane-major; planes 0-2 are packed
    # per-partition-contiguous for one efficient HWDGE copy into the slots,
    # planes 3-7 are CCE-accumulated on top (2KB-per-partition SWDGE DMAs):
    # plane v of item b: [p, c*S + s] = x[b, 8*s + v, 128*c + p]
    xb = x.astype(NPBF16)
    x8 = x.astype(NPFP8)
    xp = (x8.reshape(B, S, TPS, NK, 128).transpose(0, 2, 4, 3, 1)
          .reshape(B, TPS, 128, NK * S))
    xc = np.ascontiguousarray(xp[:, 0:3].transpose(0, 2, 1, 3))
    xa = np.ascontiguousarray(xp[:, 3:8])
    weights = _prep_weights(inputs)

    in_maps = []
    for c in range(NCORES):
        im = dict(weights)
        im["xc"] = np.ascontiguousarray(xc[c * IPC:(c + 1) * IPC])
        im["xa"] = np.ascontiguousarray(xa[c * IPC:(c + 1) * IPC])
        # var_x[p, c_, i, v] = xb[item, vids[item, v], 128*c_ + p]
        vx = np.empty((128, NK, IPC, V), NPBF16)
        for i in range(IPC):
            item = c * IPC + i
            cols = xb[item, vids[item]]          # [V, H]
            vx[:, :, i, :] = cols.reshape(V, NK, 128).transpose(2, 1, 0)
        im["var_x"] = np.ascontiguousarray(vx.reshape(128, NK * IPC * V))
        in_maps.append(im)
    return in_maps


def _get_nc(repeat=1):
    key = ("nc", repeat)
    if key not in _CACHE:
        _CACHE[key] = _build_program(repeat=repeat)
    return _CACHE[key]


def _run(inputs, trace=False, **kw):
    nc = _get_nc()
    in_maps = _make_in_maps(inputs)
    res = run_bass_kernel_spmd(nc, in_maps, list(range(NCORES)), trace=trace, **kw)
    out = np.concatenate([r["out"] for r in res.results], axis=0).astype(np.float32)
    return out, res


def make_executor(inputs, repeat=1):
    """Build the 8-core shard_map jit once and keep inputs device-resident,
    so repeated calls time dispatch + kernel execution only."""
    import jax
    from jax.sharding import Mesh, PartitionSpec
    from jax.experimental.shard_map import shard_map
    from concourse import bass2jax

    bass2jax.install_neuronx_cc_hook()
    nc = _get_nc(repeat=repeat)
    in_maps = _make_in_maps(inputs)

    import concourse.mybir as mybir_

    partition_name = nc.partition_id_tensor.name if nc.partition_id_tensor else None
    in_names, out_names, out_avals, zero_outs = [], [], [], []
    for alloc in nc.m.functions[0].allocations:
        if not isinstance(alloc, mybir_.MemoryLocationSet):
            continue
        name = alloc.memorylocations[0].name
        if alloc.kind == "ExternalInput":
            if name != partition_name:
                in_names.append(name)
        elif alloc.kind == "ExternalOutput":
            out_names.append(name)
            shape = tuple(alloc.tensor_shape)
            dtype = mybir_.dt.np(alloc.dtype)
            out_avals.append(jax.core.ShapedArray(shape, dtype))
            zero_outs.append(np.zeros(shape, dtype))
    n_params = len(in_names)
    n_outs = len(out_avals)
    all_names = in_names + out_names
    if partition_name is not None:
        all_names = all_names + [partition_name]

    def _body(*args):
        operands = list(args)
        if partition_name is not None:
            operands.append(bass2jax.partition_id_tensor())
        outs = bass2jax._bass_exec_p.bind(
            *operands,
            out_avals=tuple(out_avals),
            in_names=tuple(all_names),
            out_names=tuple(out_names),
            lowering_input_output_aliases=(),
            sim_require_finite=True,
            sim_require_nnan=True,
            nc=nc,
        )
        return tuple(outs)

    devices = jax.devices()[:NCORES]
    mesh = Mesh(np.asarray(devices), ("core",))
    sharded = jax.jit(
        shard_map(
            _body, mesh=mesh,
            in_specs=(PartitionSpec("core"),) * (n_params + n_outs),
            out_specs=(PartitionSpec("core"),) * n_outs,
            check_rep=False,
        ),
        donate_argnums=tuple(range(n_params, n_params + n_outs)),
        keep_unused=True,
    )
    from jax.sharding import NamedSharding

    sh = NamedSharding(mesh, PartitionSpec("core"))
    concat_in = [
        jax.device_put(
            np.concatenate([np.asarray(in_maps[c][nm]) for c in range(NCORES)], axis=0),
            sh,
        )
        for nm in in_names
    ]

    def run():
        zeros = [np.zeros((NCORES * z.shape[0], *z.shape[1:]), z.dtype) for z in zero_outs]
        out_arrs = sharded(*concat_in, *zeros)
        jax.block_until_ready(out_arrs)
        return np.asarray(out_arrs[0]).reshape(NCORES, IPC, S - 1).reshape(B, S - 1)

    return run


def kernel(**inputs) -> np.ndarray:
    out, _ = _run(inputs)
    return out
